# revision 7
# baseline (speedup 1.0000x reference)
"""PointPillarsScatter on 8 TRN2 NeuronCores.

Reference op: scatter N pillar feature vectors [N, 64] into a canvas
[B=4, C=64, NY=496, NX=432] at (y, x) cell coords (zero elsewhere).

Sharding: 8 cores = 4 batches x 2 y-halves. Core k=(b, g) owns the
canvas slice out[b, :, 248*g : 248*(g+1), :] -> flat [64, 107136].

Device algorithm (per core), all standard engine ops:
  - canvas is produced in column-windows of W=512 cells across 2
    column-slabs stacked on partitions: out tile [128, 512] where
    partition p = 64*a + c (a = slab, c = channel).
  - for each window, host packs the <=128 pillars that land in it into
    "slots": lhsT weights [128 slots, 128] with w[k, 64*slab_k + c] =
    feat[pillar_k, c], and a local column index idx[k] in [0, 512).
  - DVE builds onehot[k, j] = (iota[j] == idx[k]) with one tensor_scalar.
  - PE matmul lhsT.T @ onehot -> PSUM [128, 512] = the scattered window
    (empty cells read exact 0.0; occupied cells read the exact f32
    feature since onehot rows are 0/1 and products/sums are exact).
  - copy PSUM -> SBUF (DVE/ACT), DMA the window to DRAM with a 3D AP.

Self-contained: shapes hardcoded, no sibling imports.
"""

import numpy as np

NY, NX, C = 496, 432, 64
B = 4
N_CORES = 8
HALF_Y = NY // 2  # 248
CORE_COLS = HALF_Y * NX  # 107136 canvas cells per core
SLABS = 2
SLAB = CORE_COLS // SLABS  # 53568
W = 512  # window width (canvas cells per matmul)
NWIN = (SLAB + W - 1) // W  # 105 windows (last = 320 cols)
LAST_W = SLAB - (NWIN - 1) * W  # 320
SLOTS = 128  # pillar slots per matmul chunk
GROUP = 8  # weight-tile entries fetched per input DMA

_cache = {}


def _build_program(chunks_per_window, nwt, repeat=1):
    """Build the shared SPMD bass program for the given window schedule.

    chunks_per_window: list[int] of length NWIN (>=1 each), shared by all
    cores. nwt == sum(chunks_per_window) weight-tile entries.
    """
    import concourse.bacc as bacc
    import concourse.bass as bass
    import concourse.tile as tile
    import concourse.mybir as mybir
    from contextlib import ExitStack

    f32 = mybir.dt.float32

    nc = bacc.Bacc("TRN2", target_bir_lowering=False, debug=False,
                   num_devices=N_CORES)

    w_dram = nc.dram_tensor("w", [128, nwt * 128], f32, kind="ExternalInput")
    idx_dram = nc.dram_tensor("idx", [128, nwt], f32, kind="ExternalInput")
    iota_dram = nc.dram_tensor("iota", [128, W], f32, kind="ExternalInput")
    out_dram = nc.dram_tensor("out", [C, CORE_COLS], f32, kind="ExternalOutput")

    with tile.TileContext(nc) as tc, ExitStack() as ctx:
        const_pool = ctx.enter_context(tc.tile_pool(name="const", bufs=1))
        w_pool = ctx.enter_context(tc.tile_pool(name="wpool", bufs=3))
        oh_pool = ctx.enter_context(tc.tile_pool(name="ohpool", bufs=3))
        out_pool = ctx.enter_context(tc.tile_pool(name="opool", bufs=4))
        psum_pool = ctx.enter_context(
            tc.tile_pool(name="pspool", bufs=4, space="PSUM"))

        iota_t = const_pool.tile([128, W], f32)
        nc.sync.dma_start(iota_t[:], iota_dram.ap())
        idx_t = const_pool.tile([128, nwt], f32)
        nc.sync.dma_start(idx_t[:], idx_dram.ap())

        w_ap = w_dram.ap()
        n_groups = (nwt + GROUP - 1) // GROUP

        for rep in range(repeat):
          e = 0
          w_tiles = {}
          for w in range(NWIN):
            n = W if w < NWIN - 1 else LAST_W
            nchunks = chunks_per_window[w]
            ps = psum_pool.tile([128, W], f32, tag="ps")
            for t in range(nchunks):
                g = e // GROUP
                if g not in w_tiles:
                    glen = min(GROUP, nwt - g * GROUP)
                    wt = w_pool.tile([128, GROUP * 128], f32, tag="wt",
                                     name=f"wt_{rep}_{g}")
                    nc.gpsimd.dma_start(
                        wt[:, : glen * 128],
                        w_ap[:, g * GROUP * 128 : (g * GROUP + glen) * 128])
                    w_tiles[g] = wt
                wt = w_tiles[g]
                woff = (e % GROUP) * 128
                oh = oh_pool.tile([128, W], f32, tag="oh")
                nc.vector.tensor_scalar(
                    oh[:, :n], iota_t[:, :n], idx_t[:, e : e + 1], None,
                    op0=mybir.AluOpType.is_equal)
                nc.tensor.matmul(
                    ps[:, :n], wt[:, woff : woff + 128], oh[:, :n],
                    start=(t == 0), stop=(t == nchunks - 1))
                e += 1
            ot = out_pool.tile([128, W], f32, tag="ot")
            nc.vector.tensor_copy(ot[:, :n], ps[:, :n])
            # DRAM 3D AP: (slab a: stride SLAB) x (chan c: stride CORE_COLS)
            # x (col j: stride 1), at column offset w*W.
            dst = bass.AP(out_dram, w * W,
                          [[SLAB, SLABS], [CORE_COLS, C], [1, n]])
            nc.sync.dma_start(dst, ot[:, :n])
          assert e == nwt

    nc.compile()
    return nc


def _host_pack(voxel_features, coords):
    """Shard + pack inputs for the 8 cores.

    Returns (in_maps, chunks_per_window, nwt).
    """
    vf = np.ascontiguousarray(np.asarray(voxel_features, dtype=np.float32))
    cd = np.asarray(coords)
    bidx = cd[:, 0].astype(np.int64)
    yy = cd[:, 2].astype(np.int64)
    xx = cd[:, 3].astype(np.int64)

    # Per-core pillar selections and window geometry.
    cores = []
    counts_per_core = []
    for b in range(B):
        for g in range(2):
            sel = np.nonzero((bidx == b) & (yy >= g * HALF_Y)
                             & (yy < (g + 1) * HALF_Y))[0]
            flat = (yy[sel] - g * HALF_Y) * NX + xx[sel]  # [0, CORE_COLS)
            # dedupe duplicate cells, keep the LAST occurrence
            if len(flat):
                u_rev, first_rev = np.unique(flat[::-1], return_index=True)
                keep = len(flat) - 1 - first_rev
                sel, flat = sel[keep], flat[keep]
            slab = flat // SLAB
            within = flat % SLAB
            win = within // W
            loc = within % W
            # stable order by window, then assign slots within window
            order = np.argsort(win, kind="stable")
            sel, slab, win, loc = sel[order], slab[order], win[order], loc[order]
            counts = np.bincount(win, minlength=NWIN)
            starts = np.concatenate([[0], np.cumsum(counts)[:-1]])
            slot_within = np.arange(len(win)) - starts[win]
            cores.append((sel, slab, win, loc, slot_within))
            counts_per_core.append(counts)

    counts_max = np.max(np.stack(counts_per_core), axis=0)
    chunks_per_window = np.maximum(1, -(-counts_max // SLOTS)).astype(np.int64)
    nwt = int(chunks_per_window.sum())
    entry0 = np.concatenate([[0], np.cumsum(chunks_per_window)[:-1]])

    iota = np.tile(np.arange(W, dtype=np.float32), (128, 1))

    in_maps = []
    for (sel, slab, win, loc, slot_within) in cores:
        chunk = slot_within // SLOTS
        slot = (slot_within % SLOTS).astype(np.int64)
        entry = entry0[win] + chunk
        wt = np.zeros((nwt, 128, 128), dtype=np.float32)
        idxc = np.full((nwt, 128), -1.0, dtype=np.float32)
        if len(sel):
            # w[entry, slot, 64*slab + c] = vf[sel, c]
            col0 = (C * slab).astype(np.int64)
            wt[entry[:, None], slot[:, None],
               col0[:, None] + np.arange(C)[None, :]] = vf[sel]
            idxc[entry, slot] = loc.astype(np.float32)
        # device layouts: w [128(k), nwt*128(e,m)], idx [128(k), nwt(e)]
        w_dev = np.ascontiguousarray(
            wt.transpose(1, 0, 2).reshape(128, nwt * 128))
        idx_dev = np.ascontiguousarray(idxc.T)
        in_maps.append({"w": w_dev, "idx": idx_dev, "iota": iota})

    return in_maps, tuple(int(c) for c in chunks_per_window), nwt


def _run(voxel_features, coords, trace=False):
    from concourse.bass_utils import run_bass_kernel_spmd

    in_maps, chunks, nwt = _host_pack(voxel_features, coords)
    key = chunks
    if key not in _cache:
        _cache[key] = _build_program(chunks, nwt)
    nc = _cache[key]

    res = run_bass_kernel_spmd(nc, in_maps, core_ids=list(range(N_CORES)),
                               trace=trace)
    out = np.zeros((B, C, NY, NX), dtype=np.float32)
    for k in range(N_CORES):
        b, g = divmod(k, 2)
        core_out = res.results[k]["out"].reshape(C, HALF_Y, NX)
        out[b, :, g * HALF_Y : (g + 1) * HALF_Y, :] = core_out
    return out, res


def kernel(voxel_features, coords, batch_size=B):
    assert int(batch_size) == B
    out, _ = _run(voxel_features, coords, trace=False)
    return out


# revision 29
# speedup vs baseline: 38.6892x; 38.6892x over previous
"""PointPillarsScatter on 8 TRN2 NeuronCores.

Reference op: scatter N pillar feature vectors [N, 64] into a canvas
[B=4, C=64, NY=496, NX=432] at (y, x) cell coords (zero elsewhere).

Sharding: 8 cores = 4 batches x 2 y-halves. Core k=(b, g) owns the
canvas slice out[b, :, 248*g : 248*(g+1), :] -> flat [64, 107136].

Device algorithm (per core), all standard engine ops:
  - canvas is produced in column-windows of W=512 cells across 2
    column-slabs stacked on partitions: window tile [128, 512] where
    partition p = 64*a + c (a = slab, c = channel).
  - for each window, host packs the <=128 pillars that land in it into
    "slots": lhsT weights [128 slots, 128] with w[k, 64*slab_k + c] =
    feat[pillar_k, c], and a local column index idx[k] in [0, 512).
  - DVE builds onehot[k, j] = (iota[j] == idx[k]) with one tensor_scalar.
  - PE matmul lhsT.T @ onehot -> PSUM [128, 512] = the scattered window
    (empty cells read exact 0.0; occupied cells the exact f32 feature
    since onehot rows are 0/1 and products/sums are exact).
  - copy PSUM -> SBUF (alternating DVE/ACT), accumulate SUPER=8 windows
    into one [128, 4096] tile, DMA it to a CONTIGUOUS DRAM superblock
    (scattered multi-descriptor DMA patterns measured ~10x below line
    rate; contiguous superblocks merge descriptors to full rate).
  - host unscrambles superblocks into the final canvas layout.

Self-contained: shapes hardcoded, no sibling imports.
"""

import numpy as np

NY, NX, C = 496, 432, 64
B = 4
N_CORES = 8
HALF_Y = NY // 2  # 248
CORE_COLS = HALF_Y * NX  # 107136 canvas cells per core
SLABS = 2
SLAB = CORE_COLS // SLABS  # 53568
W = 512  # window width (canvas cells per matmul)
NWIN = (SLAB + W - 1) // W  # 105 windows (last = 320 cols)
LAST_W = SLAB - (NWIN - 1) * W  # 320
SLOTS = 64  # pillar slots per slab per matmul chunk (slab a owns
            # partitions [64a, 64a+64) of the slot space)
GROUP = 8  # weight-tile entries fetched per input DMA
SUPER = 8  # windows per output superblock DMA
NSB = NWIN // SUPER  # 13 full superblocks; remainder windows after that
REM_WINS = NWIN - NSB * SUPER  # 1 (the 320-col window)
OUT_ELEMS = C * CORE_COLS  # per-core output element count

_cache = {}


def _build_program(chunks_per_window, nwt, repeat=1, mode="full",
                   psum_bufs=6, oh_bufs=4, sb_bufs=3, wt_bufs=3,
                   copy_mode="alt", super_w=SUPER):
    """Build the shared SPMD bass program for the given window schedule.

    chunks_per_window: list[int] of length NWIN (>=1 each), shared by all
    cores. nwt == sum(chunks_per_window) weight-tile entries.
    mode: "full" | "dmaonly" (skip compute, DMA a constant tile) |
    "nodma" (compute, tiny out-DMA only) — bisection benchmarks.
    """
    import concourse.bacc as bacc
    import concourse.bass as bass
    import concourse.tile as tile
    import concourse.mybir as mybir
    from contextlib import ExitStack

    f32 = mybir.dt.float32

    nc = bacc.Bacc("TRN2", target_bir_lowering=False, debug=False,
                   num_devices=N_CORES)

    w_dram = nc.dram_tensor("w", [128, nwt * 128], f32, kind="ExternalInput")
    idx_dram = nc.dram_tensor("idx", [128, nwt], f32, kind="ExternalInput")
    iota_dram = nc.dram_tensor("iota", [128, W], f32, kind="ExternalInput")
    # scrambled output: NSB superblocks [128, SUPER*W] + remainder windows
    out_dram = nc.dram_tensor("out", [1, OUT_ELEMS], f32, kind="ExternalOutput")

    SUP = super_w
    NSB_L = NWIN // SUP
    with tile.TileContext(nc) as tc, ExitStack() as ctx:
        const_pool = ctx.enter_context(tc.tile_pool(name="const", bufs=1))
        w_pool = ctx.enter_context(tc.tile_pool(name="wpool", bufs=wt_bufs))
        oh_pool = ctx.enter_context(tc.tile_pool(name="ohpool", bufs=oh_bufs))
        out_pool = ctx.enter_context(tc.tile_pool(name="opool", bufs=sb_bufs))
        psum_pool = ctx.enter_context(
            tc.tile_pool(name="pspool", bufs=psum_bufs, space="PSUM"))

        iota_t = const_pool.tile([128, W], f32)
        nc.sync.dma_start(iota_t[:], iota_dram.ap())
        idx_t = const_pool.tile([128, nwt], f32)
        nc.sync.dma_start(idx_t[:], idx_dram.ap())
        zed = None
        if mode == "dmaonly":
            zed = const_pool.tile([128, SUP * W], f32)
            nc.vector.memset(zed[:], 0.125)

        w_ap = w_dram.ap()

        for rep in range(repeat):
            e = 0
            w_tiles = {}
            sb_tile = None
            sb_base = 0  # first window index of current superblock
            for w in range(NWIN):
                n = W if w < NWIN - 1 else LAST_W
                in_super = w < NSB_L * SUP
                if in_super and w % SUP == 0:
                    sb_tile = out_pool.tile([128, SUP * W], f32, tag="sb",
                                            name=f"sb_{rep}_{w // SUP}")
                    sb_base = w
                nchunks = chunks_per_window[w] if mode != "dmaonly" else 0
                ps = psum_pool.tile([128, W], f32, tag="ps",
                                    name=f"ps_{rep}_{w}")
                for t in range(nchunks):
                    g = e // GROUP
                    if g not in w_tiles:
                        glen = min(GROUP, nwt - g * GROUP)
                        wt = w_pool.tile([128, GROUP * 128], f32, tag="wt",
                                         name=f"wt_{rep}_{g}")
                        nc.gpsimd.dma_start(
                            wt[:, : glen * 128],
                            w_ap[:, g * GROUP * 128 : (g * GROUP + glen) * 128])
                        w_tiles[g] = wt
                    wt = w_tiles[g]
                    woff = (e % GROUP) * 128
                    # plain fp32 matmul (4 cycles/row): float32r runs
                    # 4x faster but is reduced precision on HW (measured
                    # absmax 1e-3) — this op must be bit-exact.
                    oh = oh_pool.tile([128, W], f32, tag="oh",
                                      name=f"oh_{rep}_{w}_{t}")
                    nc.vector.tensor_scalar(
                        oh[:, :n], iota_t[:, :n], idx_t[:, e : e + 1], None,
                        op0=mybir.AluOpType.is_equal)
                    nc.tensor.matmul(
                        ps[:, :n], wt[:, woff : woff + 128], oh[:, :n],
                        start=(t == 0), stop=(t == nchunks - 1))
                    e += 1
                if in_super:
                    j0 = (w - sb_base) * W
                    dstslice = sb_tile[:, j0 : j0 + n]
                else:
                    sb_tile = out_pool.tile([128, SUP * W], f32, tag="sb",
                                            name=f"sb_{rep}_r{w}")
                    dstslice = sb_tile[:, :n]
                if mode != "dmaonly":
                    # PSUM->SBUF copies: alternate DVE/ACT or pin one engine
                    use_v = (w % 2 == 0) if copy_mode == "alt" else (
                        copy_mode == "dve")
                    if use_v:
                        nc.vector.tensor_copy(dstslice, ps[:, :n])
                    else:
                        nc.scalar.copy(dstslice, ps[:, :n])
                if mode == "nodma":
                    off = w * 128 * 16
                    dst = bass.AP(out_dram, off, [[16, 128], [1, 16]])
                    nc.sync.dma_start(dst, sb_tile[:, :16])
                    continue
                src_tile = sb_tile if mode != "dmaonly" else zed
                if in_super and (w - sb_base) == SUP - 1:
                    off = sb_base * 128 * W
                    dst = bass.AP(out_dram, off, [[SUP * W, 128],
                                                  [1, SUP * W]])
                    nc.sync.dma_start(dst, src_tile[:])
                elif not in_super:
                    off = NSB_L * SUP * 128 * W + (w - NSB_L * SUP) * 128 * LAST_W
                    dst = bass.AP(out_dram, off, [[n, 128], [1, n]])
                    nc.sync.dma_start(dst, src_tile[:, :n])
            assert e == nwt or mode == "dmaonly"

    nc.compile()
    return nc


def _unscramble(core_flat):
    """[OUT_ELEMS] scrambled superblocks -> canvas [C, CORE_COLS]."""
    canvas = np.empty((C, CORE_COLS), dtype=np.float32)
    main = core_flat[: NSB * 128 * SUPER * W].reshape(
        NSB, SLABS, C, SUPER * W)  # [g, a, c, j]
    # canvas cols a*SLAB + g*SUPER*W + j  for j in [0, SUPER*W)
    m = main.transpose(2, 1, 0, 3).reshape(C, SLABS, NSB * SUPER * W)
    canvas_v = canvas.reshape(C, SLABS, SLAB)
    canvas_v[:, :, : NSB * SUPER * W] = m
    off = NSB * 128 * SUPER * W
    for r in range(REM_WINS):
        w = NSB * SUPER + r
        blk = core_flat[off : off + 128 * LAST_W].reshape(SLABS, C, LAST_W)
        canvas_v[:, :, w * W : w * W + LAST_W] = blk.transpose(1, 0, 2)
        off += 128 * LAST_W
    return canvas


def _host_pack(voxel_features, coords):
    """Shard + pack inputs for the 8 cores.

    Returns (in_maps, chunks_per_window, nwt).
    """
    vf = np.ascontiguousarray(np.asarray(voxel_features, dtype=np.float32))
    cd = np.asarray(coords)
    bidx = cd[:, 0].astype(np.int64)
    yy = cd[:, 2].astype(np.int64)
    xx = cd[:, 3].astype(np.int64)

    # jax scatter drops out-of-bounds indices; match by masking them out
    inb = (yy >= 0) & (yy < NY) & (xx >= 0) & (xx < NX)

    cores = []
    counts_per_core = []
    for b in range(B):
        for g in range(2):
            sel = np.nonzero(inb & (bidx == b) & (yy >= g * HALF_Y)
                             & (yy < (g + 1) * HALF_Y))[0]
            flat = (yy[sel] - g * HALF_Y) * NX + xx[sel]  # [0, CORE_COLS)
            # dedupe duplicate cells, keep the LAST occurrence
            if len(flat):
                u_rev, first_rev = np.unique(flat[::-1], return_index=True)
                keep = len(flat) - 1 - first_rev
                sel, flat = sel[keep], flat[keep]
            slab = flat // SLAB
            within = flat % SLAB
            win = within // W
            loc = within % W
            # slot space: per (window, slab); slab a owns partitions
            # [64a, 64a+64) and chunk t covers slots [64t, 64t+64) there
            key = win * SLABS + slab
            order = np.argsort(key, kind="stable")
            sel, slab, win, loc = sel[order], slab[order], win[order], loc[order]
            key = key[order]
            kcounts = np.bincount(key, minlength=NWIN * SLABS)
            starts = np.concatenate([[0], np.cumsum(kcounts)[:-1]])
            slot_within = np.arange(len(win)) - starts[key]
            cores.append((sel, slab, win, loc, slot_within))
            counts_per_core.append(kcounts)

    counts_max = np.max(np.stack(counts_per_core), axis=0).reshape(NWIN, SLABS)
    counts_max = counts_max.max(axis=1)  # worst slab per window
    chunks_per_window = np.maximum(1, -(-counts_max // SLOTS)).astype(np.int64)
    nwt = int(chunks_per_window.sum())
    entry0 = np.concatenate([[0], np.cumsum(chunks_per_window)[:-1]])

    iota = np.tile(np.arange(W, dtype=np.float32), (128, 1))

    in_maps = []
    for (sel, slab, win, loc, slot_within) in cores:
        chunk = slot_within // SLOTS
        slot = (SLOTS * slab + slot_within % SLOTS).astype(np.int64)
        entry = entry0[win] + chunk
        wt = np.zeros((nwt, 128, 128), dtype=np.float32)
        idxc = np.full((nwt, 128), -1.0, dtype=np.float32)
        if len(sel):
            col0 = (C * slab).astype(np.int64)
            wt[entry[:, None], slot[:, None],
               col0[:, None] + np.arange(C)[None, :]] = vf[sel]
            idxc[entry, slot] = loc.astype(np.float32)
        w_dev = np.ascontiguousarray(
            wt.transpose(1, 0, 2).reshape(128, nwt * 128))
        idx_dev = np.ascontiguousarray(idxc.T)
        in_maps.append({"w": w_dev, "idx": idx_dev, "iota": iota})

    return in_maps, tuple(int(c) for c in chunks_per_window), nwt


def _run(voxel_features, coords, trace=False):
    from concourse.bass_utils import run_bass_kernel_spmd

    in_maps, chunks, nwt = _host_pack(voxel_features, coords)
    key = chunks
    if key not in _cache:
        _cache[key] = _build_program(chunks, nwt)
    nc = _cache[key]

    res = run_bass_kernel_spmd(nc, in_maps, core_ids=list(range(N_CORES)),
                               trace=trace)
    out = np.zeros((B, C, NY, NX), dtype=np.float32)
    for k in range(N_CORES):
        b, g = divmod(k, 2)
        canvas = _unscramble(res.results[k]["out"].reshape(-1))
        out[b, :, g * HALF_Y : (g + 1) * HALF_Y, :] = canvas.reshape(
            C, HALF_Y, NX)
    return out, res


def kernel(voxel_features, coords, batch_size=B):
    assert int(batch_size) == B
    out, _ = _run(voxel_features, coords, trace=False)
    return out
